# revision 39
# baseline (speedup 1.0000x reference)
"""Trainium2 Bass kernel for fused attention block (nn_Attention_790273982568).

Full (unsharded) inputs in, full output out. Tensor-parallel over heads on 8
NeuronCores: each core owns 4 Q heads + 1 KV head and 512 output columns of
wo. The QKV and WO projections run as 3-term compensated fp8e4 DoubleRow
matmuls (hi/lo splits of both operands, residual products accumulated into
the same PSUM bank), attention (scores/softmax/PV) stays bf16. Phases are
fused: per-span attention starts as soon as its token tiles are projected,
per-(head,span) AllGathers overlap attention, and each span's WO matmuls are
interleaved into the following QKV/softmax pipeline.
"""

import os
import sys

import numpy as np

for _p in ("/opt/trn_rl_repo", "/root/.axon_site/_ro/trn_rl_repo"):
    if _p not in sys.path and os.path.isdir(_p):
        sys.path.append(_p)

import ml_dtypes  # noqa: E402

import concourse.bass as bass  # noqa: E402
from concourse import bacc  # noqa: E402
import concourse.mybir as mybir  # noqa: E402
import concourse.tile as tile  # noqa: E402
from concourse.bass import ds, ts  # noqa: E402
from concourse.bass_utils import run_bass_kernel_spmd  # noqa: E402

# Problem shapes (hardcoded per spec)
T = 2048
DIM = 4096
HD = 128
NH = 32
NKV = 8
NCORES = 8
QH = NH // NCORES          # 4 q heads per core
FEAT = (QH + 2) * HD       # 768 qkv features per core
OUTC = DIM // NCORES       # 512 output columns per core
P = 128
NT = T // P                # 16 token tiles
KC = DIM // P              # 32 contraction chunks
QSPAN = 512
NSPAN = T // QSPAN         # 4 query spans
WOH = 256                  # WO token half-group width
HALF = HD // 2
EPS = 1e-5
THETA = 10000.0
SCALE = 1.0 / float(np.sqrt(HD))

# fp8 pre-scales (host side); descales fold into evacuation constants
SX = 4.0
SW = 128.0
SAO = 16.0
SWO = 128.0
QKV_DESCALE = SX * SW              # xq is held at this scale (LN-invariant)
WO_DESCALE = 1.0 / (SAO * SWO)

BF16 = mybir.dt.bfloat16
F32 = mybir.dt.float32
FP8 = mybir.dt.float8e4
ALU = mybir.AluOpType
ACTF = mybir.ActivationFunctionType
DR = mybir.MatmulPerfMode.DoubleRow

_PROGRAM_CACHE = {}


def _build_body(nc, aps):
    xT8 = aps["xT8"]
    wqkvT8 = aps["wqkvT8"]
    woT8 = aps["woT8"]
    ropeP = aps["ropeP"]
    masks = aps["masks"]
    ident = aps["ident"]
    ident8 = aps["ident8"]
    ag_in = aps["ag_in"]       # [QH, NSPAN, 2, HD, QSPAN] fp8
    ag_out = aps["ag_out"]     # [QH, NSPAN, NCORES, 2, HD, QSPAN] fp8
    outT = aps["outT"]
    tc = aps["tc"]
    wb_trivial = aps["wb_trivial"]

    with (
        tc.tile_pool(name="consts", bufs=1) as consts,
        tc.tile_pool(name="wq", bufs=1) as wq_pool,
        tc.tile_pool(name="wo", bufs=1) as wo_pool,
        tc.tile_pool(name="xt", bufs=2) as xt_pool,
        tc.tile_pool(name="xq", bufs=2) as xq_pool,
        tc.tile_pool(name="st", bufs=2) as st_pool,
        tc.tile_pool(name="rt", bufs=3) as rt_pool,
        tc.tile_pool(name="rq", bufs=10) as rq_pool,
        tc.tile_pool(name="attn", bufs=2) as attn_pool,
        tc.tile_pool(name="aob", bufs=4) as aob_pool,
        tc.tile_pool(name="rr", bufs=4) as rr_pool,
        tc.tile_pool(name="aos", bufs=1) as aos_pool,
        tc.tile_pool(name="ob", bufs=2) as ob_pool,
        tc.tile_pool(name="pp", bufs=1, space="PSUM") as pp,
    ):
        def load_xt(t, eng):
            tiles = xt_pool.tile([P, KC, 2, P], FP8, tag="xt", name=f"xt_{t}")
            for s in (1, 0):  # hi first: T1 matmuls only need hi
                eng.dma_start(
                    tiles[:, :, s, :],
                    xT8[:, s, ds(t * P, P)].rearrange("(k p) c -> p k c", p=P),
                )
            return tiles

        xt_cache = {0: load_xt(0, nc.sync)}

        wqkv_sb = wq_pool.tile([P, KC, 2, FEAT], FP8, tag="wqkv8")
        for s in range(2):  # hi groups first
            for g in range(4):
                eng = nc.sync if g % 2 == 0 else nc.scalar
                eng.dma_start(
                    wqkv_sb[:, ds(8 * g, 8), s, :],
                    wqkvT8[ds(8 * g * P, 8 * P), s, :].rearrange(
                        "(k p) f -> p k f", p=P
                    ),
                )
        wo_sb = wo_pool.tile([P, KC, 2, OUTC], FP8, tag="wo8")

        ident_sb = consts.tile([P, P], BF16, tag="ident")
        nc.sync.dma_start(ident_sb[:], ident[:, :])
        ident8_sb = consts.tile([P, P], FP8, tag="ident8")
        nc.sync.dma_start(ident8_sb[:], ident8[:, :])
        masks_sb = consts.tile([P, 4, QSPAN], BF16, tag="masks")
        nc.scalar.dma_start(masks_sb[:], masks[:, :, :])
        rope_sb = consts.tile([P, NT, 2, HALF], F32, tag="rope")
        nc.scalar.dma_start(rope_sb[:], ropeP[:, :, :, :])
        if not wb_trivial:
            wb_sb = consts.tile([P, 2, 2, HD], F32, tag="wb")
            nc.sync.dma_start(wb_sb[:], aps["lnwb"][:, :, :, :])
            aps["wb_sb"] = wb_sb

        qkT = consts.tile([P, NT, 5, P], BF16, tag="qkT")
        vaug = consts.tile([P, NT, HD + 1], BF16, tag="vaug")
        nc.vector.memset(vaug[:, :, HD : HD + 1], 1.0)
        eps_sb = consts.tile([P, 1], F32, tag="eps")
        nc.vector.memset(eps_sb[:], EPS)

        fill_q = []          # deferred PE-work thunks (WO of previous span)

        def fill(n):
            for _ in range(n):
                if fill_q:
                    fill_q.pop(0)()

        tr_pend = []         # (t, [rq tiles]) awaiting PE transpose to qkT

        def drain_tr():
            while tr_pend:
                t0, rqs = tr_pend.pop(0)
                qtr = pp.tile([P, 5, P], BF16, tag="qtr", bufs=1)
                for hh in range(5):
                    nc.tensor.transpose(qtr[:, hh, :], rqs[hh][:], ident_sb[:])
                nc.scalar.copy(qkT[:, t0, :, :], qtr[:])

        def emit_qkv_chunk(t, xt_tiles, c, xq_tile):
            w0 = c * 384
            pa = pp.tile([P, 384], F32, tag="mm", bufs=2, name=f"pq_{t}_{c}")
            for ks in range(KC // 2):
                nc.tensor.matmul(
                    pa[:],
                    xt_tiles[:, ds(2 * ks, 2), 1, :],
                    wqkv_sb[:, ds(2 * ks, 2), 0, ds(w0, 384)],
                    start=(ks == 0), stop=False, perf_mode=DR,
                )
            for k in range(KC):
                nc.tensor.matmul(
                    pa[:],
                    xt_tiles[:, k, :, :],
                    wqkv_sb[:, k, :, ds(w0, 384)],
                    start=False, stop=(k == KC - 1), perf_mode=DR,
                )
            nc.scalar.copy(xq_tile[:, ds(w0, 384)], pa[:])

        def emit_ln_rope(t, xq_tile):
            sb6 = st_pool.tile([P, 5, 6], F32, tag="sb6")
            nc.vector.bn_stats(
                sb6[:, 0:4, :],
                xq_tile[:, ds(0, 4 * HD)].rearrange("p (h d) -> p h d", h=4),
            )
            nc.vector.bn_stats(sb6[:, 4:5, :], xq_tile[:, ds(4 * HD, HD)])
            mv = st_pool.tile([P, 5, 2], F32, tag="mv")
            for h in range(5):
                nc.vector.bn_aggr(mv[:, h, :], sb6[:, h, :])
            lv = st_pool.tile([P, 5], F32, tag="lv")
            nc.scalar.activation(
                lv[:], mv[:, :, 1:2].rearrange("p h one -> p (h one)"),
                ACTF.Ln, bias=eps_sb[:],
            )
            rstd = st_pool.tile([P, 5], F32, tag="rstd")
            nc.scalar.activation(rstd[:], lv[:], ACTF.Exp, scale=-0.5)
            nb = st_pool.tile([P, 5], F32, tag="nb")
            nc.vector.tensor_mul(
                nb[:], mv[:, :, 0:1].rearrange("p h one -> p (h one)"), rstd[:]
            )
            rqs = []
            cosv = rope_sb[:, t, 0, :]
            sinv = rope_sb[:, t, 1, :]
            for h in range(5):
                xh = xq_tile[:, ds(h * HD, HD)]
                xn = rt_pool.tile([P, HD], F32, tag="xn")
                nc.vector.tensor_scalar(
                    xn[:], xh, rstd[:, h : h + 1], nb[:, h : h + 1],
                    op0=ALU.mult, op1=ALU.subtract,
                )
                if not wb_trivial:
                    qk = 0 if h < QH else 1
                    nc.vector.tensor_mul(xn[:], xn[:], aps["wb_sb"][:, qk, 0, :])
                    nc.vector.tensor_add(xn[:], xn[:], aps["wb_sb"][:, qk, 1, :])
                xr = xn.rearrange("p (f two) -> p two f", two=2)
                xe = xr[:, 0, :]
                xo = xr[:, 1, :]
                ta = rt_pool.tile([P, HALF], F32, tag="ta")
                tb = rt_pool.tile([P, HALF], F32, tag="tb")
                td = rt_pool.tile([P, HALF], F32, tag="td")
                te = rt_pool.tile([P, HALF], F32, tag="te")
                nc.gpsimd.tensor_mul(ta[:], xe, cosv)
                nc.gpsimd.tensor_mul(tb[:], xo, sinv)
                nc.gpsimd.tensor_mul(td[:], xe, sinv)
                nc.gpsimd.tensor_mul(te[:], xo, cosv)
                rq = rq_pool.tile([P, HD], BF16, tag="rq", name=f"rq_{t}_{h}")
                rqr = rq.rearrange("p (f two) -> p two f", two=2)
                nc.vector.tensor_sub(rqr[:, 0, :], ta[:], tb[:])
                nc.vector.tensor_add(rqr[:, 1, :], td[:], te[:])
                rqs.append(rq)
            tr_pend.append((t, rqs))

        def emit_scores_head(h, j, nkb, pv_cb=None):
            attn = attn_pool.tile([P, NT, QSPAN], BF16, tag="attn",
                                  name=f"attn_{h}_{j}")
            for i in range(nkb):
                r = i - 4 * j
                rsub = max(0, r)
                w = QSPAN - 128 * rsub
                off = 128 * rsub
                sc = pp.tile([P, QSPAN], F32, tag="sc", bufs=2)
                nc.tensor.matmul(
                    sc[:, 0:w],
                    qkT[:, i, 4, :],
                    qkT[:, ds(4 * j + rsub, 4 - rsub), h, :],
                    start=True, stop=True,
                )
                nc.scalar.activation(
                    attn[:, i, ds(off, w)], sc[:, 0:w], ACTF.Exp, scale=SCALE
                )
                if r >= 0:
                    nc.vector.tensor_mul(
                        attn[:, i, ds(off, w)],
                        attn[:, i, ds(off, w)],
                        masks_sb[:, r, ds(off, w)],
                    )
                fill(5)
                if pv_cb is not None and i % 4 == 2:
                    pv_cb()
            return attn

        def make_pv_head(h, j, attn):
            """Returns a per-q4 emitter for head h's PV + evac + transpose."""
            aot = aob_pool.tile([P, 2, 4, P], FP8, tag="aot",
                                name=f"aot_{h}_{j}", bufs=2)

            def pv_q4(q4):
                qb = 4 * j + q4
                po = pp.tile([P, HD + 1], F32, tag="po", bufs=2)
                for i in range(qb + 1):
                    nc.tensor.matmul(
                        po[:],
                        attn[:, i, ts(q4, P)],
                        vaug[:, i, :],
                        start=(i == 0), stop=(i == qb),
                    )
                r0 = rr_pool.tile([P, 1], F32, tag="r0")
                nc.vector.reciprocal(r0[:], po[:, HD : HD + 1])
                r16 = rr_pool.tile([P, 1], F32, tag="r16")
                nc.vector.tensor_scalar_mul(r16[:], r0[:], SAO)
                hi8 = aob_pool.tile([P, HD], FP8, tag="hi8",
                                    name=f"hi8_{h}_{qb}")
                nc.scalar.activation(hi8[:], po[:, 0:HD], ACTF.Copy,
                                     scale=r16[:])
                xv = aob_pool.tile([P, HD], F32, tag="xv", bufs=2)
                nc.scalar.activation(xv[:], po[:, 0:HD], ACTF.Copy,
                                     scale=r16[:])
                lo8 = aob_pool.tile([P, HD], FP8, tag="lo8",
                                    name=f"lo8_{h}_{qb}")
                nc.vector.tensor_sub(lo8[:], xv[:], hi8[:])
                atr = pp.tile([P, 2, P], FP8, tag="atr", bufs=1,
                              name=f"atr_{h}_{qb}")
                nc.tensor.transpose(atr[:, 0, :], lo8[:], ident8_sb[:])
                nc.tensor.transpose(atr[:, 1, :], hi8[:], ident8_sb[:])
                nc.vector.tensor_copy(aot[:, :, q4, :], atr[:])
                fill(3)
                if q4 == 3:
                    nc.sync.dma_start(
                        ag_in[j, h],
                        aot[:].rearrange("p two q c -> two p (q c)"),
                    )
                    emit_ag(j, h)

            return pv_q4

        def emit_ag(j, h):
            if aps.get("no_collective"):
                for rnk in range(NCORES):
                    nc.sync.dma_start(ag_out[j, h, rnk], ag_in[j, h])
            else:
                nc.gpsimd.collective_compute(
                    "AllGather",
                    ALU.bypass,
                    replica_groups=[list(range(NCORES))],
                    ins=[ag_in[j, h]],
                    outs=[ag_out[j, h]],
                )



        def emit_wo_span(j):
            """Queue WO matmuls for token span j into fill_q.

            Chunk k = 4r + h (global head order). WO accumulation is ordered
            heads {0,1} first, {2,3} last, so the last head's AllGather
            latency hides behind the earlier heads' matmuls.
            """
            for half in range(2):
                ao_sb = aos_pool.tile([P, KC, 2, WOH], FP8, tag="ao_sb",
                                      name=f"ao_{j}_{half}")
                ao_rh = ao_sb.rearrange("p (r hh) two t -> p r hh two t", hh=QH)

                def mk_load(h, s, ao_rh=ao_rh, half=half):
                    def f():
                        nc.scalar.dma_start(
                            ao_rh[:, :, h, s, :],
                            ag_out[j, h, :, s, :, ds(half * WOH, WOH)].rearrange(
                                "r p t -> p r t"
                            ),
                        )
                    return f

                pw_box = {}

                def mk_alloc(cb, half=half):
                    def f():
                        pw_box[cb] = pp.tile(
                            [P, WOH], F32, tag="mm", bufs=2,
                            name=f"pw_{j}_{half}_{cb}",
                        )
                    return f

                def mk_mm(cb, kind, k, ao_sb=ao_sb):
                    def f():
                        pw = pw_box[cb]
                        if kind == 0:
                            nc.tensor.matmul(
                                pw[:],
                                wo_sb[:, ds(2 * k, 2), 0, ts(cb, P)],
                                ao_sb[:, ds(2 * k, 2), 1, :],
                                start=(k == 0), stop=False, perf_mode=DR,
                            )
                        else:
                            nc.tensor.matmul(
                                pw[:],
                                wo_sb[:, k, :, ts(cb, P)],
                                ao_sb[:, k, :, :],
                                start=False, stop=(k == KC - 1), perf_mode=DR,
                            )
                    return f

                ob_box = {}

                def mk_evac(cb, half=half):
                    def f():
                        if cb == 0:
                            ob_box[0] = ob_pool.tile(
                                [P, 4, WOH], BF16, tag="ob",
                                name=f"ob_{j}_{half}",
                            )
                        ob = ob_box[0]
                        nc.scalar.activation(ob[:, cb, :], pw_box[cb][:],
                                             ACTF.Copy, scale=WO_DESCALE)
                        if cb == 3:
                            nc.sync.dma_start(
                                outT[:, ds(j * QSPAN + half * WOH, WOH)].rearrange(
                                    "(cb p) t -> p cb t", p=P
                                ),
                                ob[:],
                            )
                    return f

                # T1 pair-step ks covers chunks (2ks, 2ks+1) = rank ks//2,
                # heads (0,1) when ks is even, heads (2,3) when odd.
                t1_01 = [ks for ks in range(KC // 2) if ks % 2 == 0]
                t1_23 = [ks for ks in range(KC // 2) if ks % 2 == 1]
                cross = {h: [4 * r + h for r in range(NCORES)] for h in range(QH)}

                def part1(cb):
                    return ([mk_mm(cb, 0, ks) for ks in t1_01]
                            + [mk_mm(cb, 1, k) for k in cross[0]]
                            + [mk_mm(cb, 1, k) for k in cross[1]])

                def part2(cb):
                    return ([mk_mm(cb, 0, ks) for ks in t1_23]
                            + [mk_mm(cb, 1, k) for k in cross[2]]
                            + [mk_mm(cb, 1, k) for k in cross[3]])

                for h in (0, 1):
                    for s in range(2):
                        fill_q.append(mk_load(h, s))
                for cbp in (0, 2):
                    fill_q.append(mk_alloc(cbp))
                    fill_q.extend(part1(cbp))
                    if cbp == 0:
                        for h in (2, 3):
                            for s in range(2):
                                fill_q.append(mk_load(h, s))
                    fill_q.append(mk_alloc(cbp + 1))
                    fill_q.extend(part1(cbp + 1))
                    fill_q.extend(part2(cbp))
                    fill_q.append(mk_evac(cbp))
                    fill_q.extend(part2(cbp + 1))
                    fill_q.append(mk_evac(cbp + 1))

        class _PVEmitter:
            def __init__(self, fn):
                self.fn = fn
                self.q = 0

            def next(self):
                if self.q < 4:
                    self.fn(self.q)
                    self.q += 1

            def drain(self):
                while self.q < 4:
                    self.next()

        def emit_span(j):
            nkb = 4 * (j + 1)
            pv_prev = None
            for h in range(QH):
                attn = emit_scores_head(
                    h, j, nkb, pv_cb=(pv_prev.next if pv_prev else None)
                )
                if pv_prev is not None:
                    pv_prev.drain()
                pv_prev = _PVEmitter(make_pv_head(h, j, attn))
            pv_prev.drain()
            emit_wo_span(j)

        # ---------------- main token-tile loop ----------------
        for t in range(NT):
            if t + 1 < NT:
                xt_cache[t + 1] = load_xt(t + 1, nc.sync)
            if t < 4:
                s, gg = divmod(t, 2)
                nc.scalar.dma_start(
                    wo_sb[:, ds(16 * gg, 16), s, :],
                    woT8[ds(16 * gg * P, 16 * P), s, :].rearrange(
                        "(k p) f -> p k f", p=P
                    ),
                )
            xt_tiles = xt_cache.pop(t)
            xq_tile = xq_pool.tile([P, FEAT], BF16, tag="xq", name=f"xq_{t}")
            emit_qkv_chunk(t, xt_tiles, 0, xq_tile)
            fill(12)
            if t > 0:
                drain_tr()
            emit_qkv_chunk(t, xt_tiles, 1, xq_tile)
            fill(12)
            nc.vector.tensor_scalar_mul(
                vaug[:, t, 0:HD], xq_tile[:, ds(640, HD)], 1.0 / QKV_DESCALE
            )
            emit_ln_rope(t, xq_tile)
            if t % 4 == 3:
                fill(30)
                drain_tr()
                emit_span(t // 4)

        while fill_q:
            fill(1)


def _build_program(no_collective=False, wb_trivial=True):
    nc = bacc.Bacc(
        "TRN2",
        target_bir_lowering=False,
        debug=False,
        enable_asserts=True,
        num_devices=1 if no_collective else NCORES,
    )
    aps = {
        "xT8": nc.dram_tensor("xT8", [DIM, 2, T], FP8, kind="ExternalInput").ap(),
        "wqkvT8": nc.dram_tensor(
            "wqkvT8", [DIM, 2, FEAT], FP8, kind="ExternalInput"
        ).ap(),
        "woT8": nc.dram_tensor(
            "woT8", [NH * HD, 2, OUTC], FP8, kind="ExternalInput"
        ).ap(),
        "ropeP": nc.dram_tensor(
            "ropeP", [P, NT, 2, HALF], F32, kind="ExternalInput"
        ).ap(),
        "lnwb": nc.dram_tensor("lnwb", [P, 2, 2, HD], F32, kind="ExternalInput").ap(),
        "masks": nc.dram_tensor("masks", [P, 4, QSPAN], BF16, kind="ExternalInput").ap(),
        "ident": nc.dram_tensor("ident", [P, P], BF16, kind="ExternalInput").ap(),
        "ident8": nc.dram_tensor("ident8", [P, P], FP8, kind="ExternalInput").ap(),
        "ag_in": nc.dram_tensor("ag_in", [NSPAN, QH, 2, HD, QSPAN], FP8).ap(),
        "ag_out": nc.dram_tensor(
            "ag_out", [NSPAN, QH, NCORES, 2, HD, QSPAN], FP8, addr_space="Shared"
        ).ap(),
        "outT": nc.dram_tensor("outT", [OUTC, T], BF16, kind="ExternalOutput").ap(),
    }
    aps["no_collective"] = no_collective
    aps["wb_trivial"] = wb_trivial
    with tile.TileContext(nc) as tc:
        aps["tc"] = tc
        _build_body(nc, aps)
    nc.compile()
    return nc


def get_program(no_collective=False, wb_trivial=True):
    key = (no_collective, wb_trivial)
    if key not in _PROGRAM_CACHE:
        _PROGRAM_CACHE[key] = _build_program(*key)
    return _PROGRAM_CACHE[key]


def _rope_tables():
    """cos/sin tables computed exactly like the reference (jax fp32 on cpu)."""
    try:
        import jax

        cpu = jax.devices("cpu")[0]
        with jax.default_device(cpu):
            import jax.numpy as jnp

            inv_freq = 1.0 / (
                THETA ** (jnp.arange(HALF, dtype=jnp.float32) * 2.0 / HD)
            )
            pos = jnp.arange(T, dtype=jnp.float32)
            ang = pos[:, None] * inv_freq[None, :]
            cos = np.asarray(jnp.cos(ang), dtype=np.float32)
            sin = np.asarray(jnp.sin(ang), dtype=np.float32)
    except Exception:
        inv_freq = (
            1.0 / (THETA ** (np.arange(HALF, dtype=np.float32) * 2.0 / HD))
        ).astype(np.float32)
        ang = np.arange(T, dtype=np.float32)[:, None] * inv_freq[None, :]
        cos = np.cos(ang).astype(np.float32)
        sin = np.sin(ang).astype(np.float32)
    return cos, sin


def _split8(x):
    hi = x.astype(ml_dtypes.float8_e4m3)
    lo = (x - hi.astype(np.float32)).astype(ml_dtypes.float8_e4m3)
    return hi, lo


def _make_const_inputs(q_ln_w, q_ln_b, k_ln_w, k_ln_b):
    cos, sin = _rope_tables()  # [T, HALF] f32
    ropeP = np.zeros((P, NT, 2, HALF), np.float32)
    ropeP[:, :, 0] = cos.reshape(NT, P, HALF).transpose(1, 0, 2)
    ropeP[:, :, 1] = sin.reshape(NT, P, HALF).transpose(1, 0, 2)

    lnwb = np.zeros((P, 2, 2, HD), np.float32)
    lnwb[:, 0, 0] = np.asarray(q_ln_w, np.float32)[None, :]
    lnwb[:, 0, 1] = np.asarray(q_ln_b, np.float32)[None, :]
    lnwb[:, 1, 0] = np.asarray(k_ln_w, np.float32)[None, :]
    lnwb[:, 1, 1] = np.asarray(k_ln_b, np.float32)[None, :]

    f = np.arange(QSPAN)[None, None, :]
    r = np.arange(4)[None, :, None]
    p = np.arange(P)[:, None, None]
    masks = (f >= 128 * r + p).astype(ml_dtypes.bfloat16)  # [P, 4, QSPAN]
    ident = np.eye(P, dtype=ml_dtypes.bfloat16)
    ident8 = np.eye(P, dtype=ml_dtypes.float8_e4m3)
    return ropeP, lnwb, masks, ident, ident8


# WO k-chunks arrive in ag_out order (rank, head-of-rank) = (r, h), and the
# global head of rank r's h-th head is 4r + h = the chunk index itself, so
# woT rows need no permutation.


def make_in_maps(inputs):
    x = np.asarray(inputs["x"], dtype=ml_dtypes.bfloat16).astype(np.float32)
    wqkv = np.asarray(inputs["wqkv"], dtype=ml_dtypes.bfloat16).astype(np.float32)
    wo = np.asarray(inputs["wo"], dtype=ml_dtypes.bfloat16).astype(np.float32)
    q_ln_w = np.asarray(inputs["q_ln_w"], np.float32)
    q_ln_b = np.asarray(inputs["q_ln_b"], np.float32)
    k_ln_w = np.asarray(inputs["k_ln_w"], np.float32)
    k_ln_b = np.asarray(inputs["k_ln_b"], np.float32)

    ropeP, lnwb, masks, ident, ident8 = _make_const_inputs(
        q_ln_w, q_ln_b, k_ln_w, k_ln_b
    )

    # x: [T, DIM] -> xT8 [DIM, 2(lo,hi), T]
    xh, xl = _split8(SX * x.T)
    xT8 = np.ascontiguousarray(np.stack([xl, xh], axis=1))

    in_maps = []
    for c in range(NCORES):
        qrows = wqkv[c * QH * HD : (c + 1) * QH * HD]
        krows = wqkv[NH * HD + c * HD : NH * HD + (c + 1) * HD]
        vrows = wqkv[(NH + NKV) * HD + c * HD : (NH + NKV) * HD + (c + 1) * HD]
        wq_c = np.concatenate([qrows, krows, vrows], axis=0).T  # [DIM, FEAT]
        wh, wl = _split8(SW * wq_c)
        wqkvT8_c = np.ascontiguousarray(np.stack([wh, wl], axis=1))

        wo_c = wo[c * OUTC : (c + 1) * OUTC, :].T  # [NH*HD, OUTC]
        oh, ol = _split8(SWO * wo_c)
        woT8_c = np.ascontiguousarray(np.stack([oh, ol], axis=1))

        in_maps.append(
            {
                "xT8": xT8,
                "wqkvT8": wqkvT8_c,
                "woT8": woT8_c,
                "ropeP": ropeP,
                "lnwb": lnwb,
                "masks": masks,
                "ident": ident,
                "ident8": ident8,
            }
        )
    return in_maps


def _wb_trivial(inputs):
    return bool(
        np.all(np.asarray(inputs["q_ln_w"], np.float32) == 1.0)
        and np.all(np.asarray(inputs["k_ln_w"], np.float32) == 1.0)
        and np.all(np.asarray(inputs["q_ln_b"], np.float32) == 0.0)
        and np.all(np.asarray(inputs["k_ln_b"], np.float32) == 0.0)
    )


def kernel(**inputs):
    nc = get_program(wb_trivial=_wb_trivial(inputs))
    in_maps = make_in_maps(inputs)
    res = run_bass_kernel_spmd(nc, in_maps, list(range(NCORES)))
    outT_full = np.concatenate(
        [np.asarray(res.results[c]["outT"]) for c in range(NCORES)], axis=0
    )
    return np.ascontiguousarray(outT_full.T).astype(ml_dtypes.bfloat16)


if __name__ == "__main__":
    nc = get_program(no_collective=True)
    print("program built ok")
